# revision 2
# baseline (speedup 1.0000x reference)
"""Trainium2 Bass kernel for EnhancedGATModel (3-layer GATv2, N=50000, E=800000).

v2 design (8 cores, dst-partitioned, degree-sorted node permutation):
- bf16 gather tables + edge math, f32 PSUM accumulation.
- L0/L1: dense edge tiles (128 edges/tile) + one-hot scatter matmuls on PE.
  Gather streams reordered [8-block superblock: all-lo | all-hi] so xl
  gathers run in full 1024-index chunks; xr gathered from local table.
- L2: degree-bucketed (partition = dst node, slots = edges) - no one-hot,
  no xr gather, slot-reduce on DVE.
- dma_gather prepare_only + trigger_dma to overlap desc-gen with transfers.
- AllGathers in bf16; h/hT/xr2/outputs SBUF-resident; batched posts.
"""
import sys
import numpy as np

sys.path.insert(0, "/opt/trn_rl_repo")

import ml_dtypes
import concourse.bass as bass
import concourse.mybir as mybir
import concourse.tile as tile
from concourse import bacc
from concourse.bass_utils import run_bass_kernel_spmd

F32 = mybir.dt.float32
BF16 = mybir.dt.float16
I16 = mybir.dt.int16
AF = mybir.ActivationFunctionType
ALU = mybir.AluOpType
BF = np.float16

NCORES = 8
N = 50000
NPC = N // NCORES          # 6250
HALFN = N // 2             # 25000
D_IN, HID, HEADS, OUT = 128, 64, 4, 2
HC = HEADS * HID           # 256
NEG_SLOPE = 0.2
BN_EPS = 1e-5
NBLK = 49                  # 48 full 128-node blocks + one 106-node block
SB = 3                     # superblock: blocks sharing a lo/hi gather stream (PSUM-bound)
GMAX = 8                   # dma_gather limit: 1024 idx = 8 tiles of 128
PREP = False                # prepare_only + trigger_dma pipelining


# ---------------------------------------------------------------- host prep
def _wrap_idx(A):
    """[C,128,TT] (slot, tile) -> [C,128,8TT] gather index layout."""
    C, P, TT = A.shape
    B = A.transpose(0, 2, 1).reshape(C, TT, 8, 16).transpose(0, 3, 1, 2)
    return np.tile(B.reshape(C, 16, 8 * TT), (1, 8, 1)).astype(np.int16)


def preprocess(edge_index):
    src = np.concatenate([edge_index[0], np.arange(N)]).astype(np.int64)
    dst = np.concatenate([edge_index[1], np.arange(N)]).astype(np.int64)
    E2 = len(src)

    deg = np.bincount(dst, minlength=N)
    rank_order = np.argsort(-deg, kind="stable")      # rank -> node
    rr = np.empty(N, np.int64)
    rr[rank_order] = np.arange(N)                      # node -> rank
    r = np.arange(N)
    c_of_r = np.where(r < 49152, (r % 1024) // 128, (r - 49152) // 106)
    pos_of_r = np.where(r < 49152, (r // 1024) * 128 + r % 128,
                        6144 + (r - 49152) % 106)
    c_of = c_of_r[rr]                                  # node -> core
    pos_of = pos_of_r[rr]                              # node -> row in core
    gpr = c_of * NPC + pos_of                          # node -> global row
    node_at = np.empty(N, np.int64)
    node_at[gpr] = np.arange(N)                        # global row -> node

    cd = c_of[dst]
    posd = pos_of[dst]
    bd = posd // 128
    pd = posd % 128
    gs = gpr[src]
    half = (gs >= HALFN).astype(np.int64)
    sidx = gs - half * HALFN                           # int16-safe

    # ---------------- dense layout (L0/L1) ----------------
    cnt = np.zeros((NCORES, NBLK, 2), np.int64)
    np.add.at(cnt, (cd, bd, half), 1)
    mx = cnt.max(axis=0)
    T = -(-mx // 128)                                  # [NBLK,2] tiles

    order = []
    for s0 in range(0, NBLK, SB):
        bs = list(range(s0, min(s0 + SB, NBLK)))
        for h in (0, 1):
            for b in bs:
                order.append((b, h))
    tile_base = {}
    t0 = 0
    for (b, h) in order:
        tile_base[(b, h)] = t0
        t0 += T[b, h]
    TT = t0
    tb_arr = np.zeros((NBLK, 2), np.int64)
    for (b, h), v in tile_base.items():
        tb_arr[b, h] = v

    key = (cd * NBLK + bd) * 2 + half
    si = np.argsort(key, kind="stable")
    ks = key[si]
    starts = np.searchsorted(ks, np.arange(NCORES * NBLK * 2))
    rin = np.arange(E2) - starts[ks]
    cs, bs_, hs = cd[si], bd[si], half[si]
    tile_of = tb_arr[bs_, hs] + rin // 128
    slot = rin % 128

    Sxl = np.zeros((NCORES, 128, TT), np.int64)
    Sxl[cs, slot, tile_of] = sidx[si]
    Sxr = np.zeros((NCORES, 128, TT), np.int64)
    Sxr[cs, slot, tile_of] = posd[si]
    Dst = np.full((NCORES, 128, TT), 200.0, np.float32)
    Dst[cs, slot, tile_of] = pd[si]

    idx_xl = _wrap_idx(Sxl)
    idx_xr = _wrap_idx(Sxr)
    dstl = Dst.astype(BF)

    # xl gather chunks (per same-half run in stream order) + xr chunks
    xl_chunks = []
    for s0 in range(0, NBLK, SB):
        bs2 = list(range(s0, min(s0 + SB, NBLK)))
        for h in (0, 1):
            run0 = tile_base[(bs2[0], h)]
            runT = sum(int(T[b, h]) for b in bs2)
            t = run0
            while t < run0 + runT:
                n = min(GMAX, run0 + runT - t)
                xl_chunks.append((h, t, n))
                t += n
    blk_range = {b: [(tile_base[(b, 0)], int(T[b, 0])),
                     (tile_base[(b, 1)], int(T[b, 1]))] for b in range(NBLK)}

    # ---------------- bucketed layout (L2) ----------------
    key2 = key * 128 + pd
    si2 = np.argsort(key2, kind="stable")
    k2s = key2[si2]
    starts2 = np.searchsorted(k2s, np.arange(NCORES * NBLK * 2 * 128))
    rin2 = np.arange(E2) - starts2[k2s]                # slot within (c,b,h,p)
    cnt2 = np.zeros((NCORES, NBLK, 2, 128), np.int64)
    np.add.at(cnt2, (cd, bd, half, pd), 1)
    S2 = cnt2.max(axis=(0, 3))                         # [NBLK,2] slots

    base2 = {}
    t0 = 0
    for s0 in range(0, NBLK, SB):
        bs2 = list(range(s0, min(s0 + SB, NBLK)))
        for h in (0, 1):
            for b in bs2:
                base2[(b, h)] = t0
                t0 += int(S2[b, h])
    TT2 = t0
    b2_arr = np.zeros((NBLK, 2), np.int64)
    for (b, h), v in base2.items():
        b2_arr[b, h] = v

    c2, b2, h2, p2 = cd[si2], bd[si2], half[si2], pd[si2]
    tile2 = b2_arr[b2, h2] + rin2
    Sx2 = np.zeros((NCORES, 128, TT2), np.int64)
    Sx2[c2, p2, tile2] = sidx[si2]
    Msk = np.full((NCORES, 128, TT2), -30.0, np.float32)
    Msk[c2, p2, tile2] = 0.0
    idx2 = _wrap_idx(Sx2)
    mask2 = Msk.astype(BF)

    l2_chunks = []
    for s0 in range(0, NBLK, SB):
        bs2 = list(range(s0, min(s0 + SB, NBLK)))
        for h in (0, 1):
            run0 = base2[(bs2[0], h)]
            runT = sum(int(S2[b, h]) for b in bs2)
            t = run0
            while t < run0 + runT:
                n = min(GMAX, run0 + runT - t)
                l2_chunks.append((h, t, n))
                t += n
    blk2_range = {b: [(base2[(b, 0)], int(S2[b, 0])),
                      (base2[(b, 1)], int(S2[b, 1]))] for b in range(NBLK)}

    meta = dict(TT=TT, TT2=TT2, xl_chunks=xl_chunks,
                l2_chunks=l2_chunks, blk_range=blk_range, blk2_range=blk2_range,
                order=order)
    return idx_xl, idx_xr, dstl, idx2, mask2, node_at, meta


def pack_consts(ip):
    """[128, CW] bf16 const pack. Returns (array, slices dict)."""
    cols = {}
    parts = []
    c0 = [0]

    def add(name, arr):
        arr = np.asarray(arr, np.float32)
        a = np.zeros((128, arr.shape[1]), np.float32)
        a[:arr.shape[0]] = arr
        cols[name] = (arr.shape[0], c0[0], arr.shape[1])
        parts.append(a)
        c0[0] += arr.shape[1]

    bcast = lambda v: np.broadcast_to(
        np.asarray(v, np.float32).reshape(-1)[None, :], (128, np.asarray(v).size)).copy()

    iota = np.broadcast_to(np.arange(128, dtype=np.float32), (128, 128))
    add("iota", np.ascontiguousarray(iota))
    add("iotaC", np.arange(128, dtype=np.float32)[:, None])
    add("attB0", bcast(ip["att0"]))
    add("attB1", bcast(ip["att1"]))
    add("attB2", bcast(ip["att2"]))
    g, bt = np.asarray(ip["bn_gamma"]), np.asarray(ip["bn_beta"])
    mu, var = np.asarray(ip["bn_mean"]), np.asarray(ip["bn_var"])
    for l in range(2):
        a = g[l] / np.sqrt(var[l] + BN_EPS)
        b = bt[l] - mu[l] * a + a * np.asarray(ip[f"bias{l}"], np.float32)
        add(f"aB{l}", bcast(a))
        add(f"bB{l}", bcast(b))
    add("bias2B", bcast(ip["bias2"]))
    add("W_in", np.asarray(ip["W_in"], np.float32))
    add("b_in", np.asarray(ip["b_in"], np.float32).reshape(-1, 1))
    add("Wl0", np.asarray(ip["Wl0"], np.float32))
    add("Wr0", np.asarray(ip["Wr0"], np.float32))
    for nm in ("Wl1", "Wr1"):
        W = np.asarray(ip[nm], np.float32)
        add(nm + "k0", W[:128])
        add(nm + "k1", W[128:])
    for nm in ("Wl2", "Wr2"):
        W = np.asarray(ip[nm], np.float32)
        add(nm + "k0", W[:128])
        add(nm + "k1", W[128:])
    return np.concatenate(parts, axis=1).astype(BF), cols


# ---------------------------------------------------------------- device
COLS = None


def build(meta, CW):
    TT, TT2 = meta["TT"], meta["TT2"]
    xl_chunks = meta["xl_chunks"]
    l2_chunks = meta["l2_chunks"]
    blk_range, blk2_range = meta["blk_range"], meta["blk2_range"]

    nc = bacc.Bacc("TRN2", target_bir_lowering=False, debug=False,
                   num_swdge_queues=1)

    xT = nc.dram_tensor("xT", [D_IN, NPC], BF16, kind="ExternalInput")
    constsF = nc.dram_tensor("constsF", [128, 2], F32, kind="ExternalInput")
    idx_xl = nc.dram_tensor("idx_xl", [128, 8 * TT], I16, kind="ExternalInput")
    idx_xr = nc.dram_tensor("idx_xr", [128, 8 * TT], I16, kind="ExternalInput")
    dstl = nc.dram_tensor("dstl", [128, TT], BF16, kind="ExternalInput")
    idx2 = nc.dram_tensor("idx2", [128, 8 * TT2], I16, kind="ExternalInput")
    mask2 = nc.dram_tensor("mask2", [128, TT2], BF16, kind="ExternalInput")
    consts = nc.dram_tensor("consts", [128, CW], BF16, kind="ExternalInput")
    out = nc.dram_tensor("out", [NPC, OUT], F32, kind="ExternalOutput")

    xl0_own = nc.dram_tensor("xl0_own", [NPC, HC], BF16)
    xl0_full = nc.dram_tensor("xl0_full", [N, HC], BF16, addr_space="Shared")
    xr0 = nc.dram_tensor("xr0", [NPC, HC], BF16)
    xl1_own = nc.dram_tensor("xl1_own", [NPC, HC], BF16)
    xl1_full = nc.dram_tensor("xl1_full", [N, HC], BF16, addr_space="Shared")
    xr1 = nc.dram_tensor("xr1", [NPC, HC], BF16)
    xl2_own = nc.dram_tensor("xl2_own", [NPC, 128], BF16)
    xl2_full = nc.dram_tensor("xl2_full", [N, 128], BF16, addr_space="Shared")

    rg = [list(range(NCORES))]
    chunk_starts = list(range(0, NPC, 128))  # 49 chunks, last 106 wide

    with tile.TileContext(nc) as tc:
        import contextlib
        with contextlib.ExitStack() as ctx:
            cst = ctx.enter_context(tc.tile_pool(name="cst", bufs=1))
            per = ctx.enter_context(tc.tile_pool(name="per", bufs=1))
            sb = ctx.enter_context(tc.tile_pool(name="sb", bufs=2))
            gat = ctx.enter_context(tc.tile_pool(name="gat", bufs=4))
            ps = ctx.enter_context(tc.tile_pool(name="ps", bufs=1, space="PSUM"))
            psn = ctx.enter_context(tc.tile_pool(name="psn", bufs=1, space="PSUM"))

            C = cst.tile([128, CW], BF16)
            nc.sync.dma_start(C[:], consts[:])
            CF = cst.tile([128, 2], F32)
            nc.sync.dma_start(CF[:], constsF[:])
            es_idx = contextlib.ExitStack()
            idx01 = es_idx.enter_context(tc.tile_pool(name="idx01", bufs=1))
            ixl_t = idx01.tile([128, 8 * TT], I16)
            nc.sync.dma_start(ixl_t[:], idx_xl[:])
            ixr_t = idx01.tile([128, 8 * TT], I16)
            nc.sync.dma_start(ixr_t[:], idx_xr[:])
            dstl_t = idx01.tile([128, TT], BF16)
            nc.sync.dma_start(dstl_t[:], dstl[:])

            def cs(name):
                r, c0i, w = COLS[name]
                return C[0:r, c0i:c0i + w]

            ident = cst.tile([128, 128], BF16)
            nc.vector.tensor_scalar(out=ident[:], in0=cs("iota"),
                                    scalar1=CF[:, 0:1],
                                    scalar2=None, op0=ALU.is_equal)

            # persistent SBUF state
            h_res = per.tile([128, NBLK, HC], BF16)      # residual (h1)
            hT0 = per.tile([128, NBLK * 128], BF16)      # h^T channels 0:128
            hT1 = per.tile([128, NBLK * 128], BF16)      # h^T channels 128:256
            xr2_sb = per.tile([128, NBLK, OUT], BF16)    # L2 dst transform
            o_all = per.tile([128, NBLK, OUT], F32)      # L2 outputs pre-softmax

            if PREP:
                dsems = [nc.alloc_semaphore("dsem0")]
            qrr = [0]

            def gather(g_ap, table_ap, idx_tile, t0, Tc, elem, qn=None, sem=None):
                if PREP:
                    q = 0
                    nc.gpsimd.dma_gather(
                        out_ap=g_ap, in_ap=table_ap,
                        idxs_ap=idx_tile[:, 8 * t0:8 * (t0 + Tc)],
                        num_idxs=128 * Tc, num_idxs_reg=128 * Tc,
                        elem_size=elem, prepare_only=True, sem=dsems[q],
                        queue_num=q)
                    nc.gpsimd.trigger_dma(count=None, queue_num=q)
                else:
                    nc.gpsimd.dma_gather(
                        out_ap=g_ap, in_ap=table_ap,
                        idxs_ap=idx_tile[:, 8 * t0:8 * (t0 + Tc)],
                        num_idxs=128 * Tc, num_idxs_reg=128 * Tc,
                        elem_size=elem)

            # ---------------- phase A: L0 node prep ----------------
            def flush_stage(tab, stage, g0, width):
                rows = min(8 * 128, NPC - g0 * 128)
                if rows == 1024:
                    nc.sync.dma_start(
                        tab[g0 * 128:g0 * 128 + 1024, :].rearrange(
                            "(t p) c -> p t c", p=128), stage[:])
                else:
                    full = rows // 128
                    for k in range(full):
                        nc.sync.dma_start(
                            tab[g0 * 128 + k * 128:g0 * 128 + (k + 1) * 128, :],
                            stage[:, k, :])
                    rem = rows - full * 128
                    if rem:
                        nc.sync.dma_start(
                            tab[g0 * 128 + full * 128:g0 * 128 + rows, :],
                            stage[:rem, full, :])

            for g0 in range(0, NBLK, 8):
                gn = min(8, NBLK - g0)
                stageL = sb.tile([128, 8, HC], BF16, tag="stgL", bufs=1)
                stageR = sb.tile([128, 8, HC], BF16, tag="stgR", bufs=1)
                for k in range(gn):
                    st = (g0 + k) * 128
                    szk = min(128, NPC - st)
                    xTc = sb.tile([D_IN, 128], BF16, tag="xTc")
                    nc.sync.dma_start(xTc[:, :szk], xT[:, st:st + szk])
                    p1 = psn.tile([64, 128], F32, tag="p1", space="PSUM")
                    nc.tensor.matmul(p1[:, :szk], lhsT=cs("W_in"), rhs=xTc[:, :szk],
                                     start=True, stop=True)
                    h0T = sb.tile([64, 128], BF16, tag="h0T")
                    nc.scalar.activation(h0T[:, :szk], p1[:, :szk], AF.Relu,
                                         bias=CF[0:64, 1:2])
                    for W, stg in (("Wl0", stageL), ("Wr0", stageR)):
                        p2 = psn.tile([128, HC], F32, tag="p2", space="PSUM")
                        nc.tensor.matmul(p2[:szk, :], lhsT=h0T[:, :szk],
                                         rhs=cs(W), start=True, stop=True)
                        nc.scalar.copy(stg[:szk, k, :], p2[:szk, :])
                flush_stage(xl0_own, stageL, g0, HC)
                flush_stage(xr0, stageR, g0, HC)

            nc.gpsimd.collective_compute(
                "AllGather", ALU.bypass, ins=[xl0_own[:]], outs=[xl0_full[:]],
                replica_groups=rg)

            # ---------------- dense edge pass (L0/L1) ----------------
            def edge_pass(xl_full, xr_tab, attB, aB, bB, residual, layer):
                # matmul bookkeeping per block
                first_tile = {}
                last_tile = {}
                for b in range(NBLK):
                    (l0, lT), (h0_, hT_) = blk_range[b]
                    tiles = list(range(l0, l0 + lT)) + list(range(h0_, h0_ + hT_))
                    if tiles:
                        first_tile[b] = min(tiles)
                        # stop on the hi-run end (hi comes later in stream)
                        last_tile[b] = max(tiles)
                tile_to_block = {}
                for b in range(NBLK):
                    for (t0, Tn) in blk_range[b]:
                        for t in range(t0, t0 + Tn):
                            tile_to_block[t] = b
                accs = {}

                def post(b):
                    acc = accs.pop(b)
                    st = b * 128
                    nreal = min(128, NPC - st)
                    rc = sb.tile([128, HEADS, 1], F32, tag="rc")
                    nc.vector.reciprocal(rc[:], acc[:, :, HID:HID + 1])
                    go = sb.tile([128, HC], BF16, tag="go")
                    nc.vector.tensor_tensor(
                        out=go[:].rearrange("a (h c) -> a h c", h=HEADS),
                        in0=acc[:, :, 0:HID],
                        in1=rc[:].to_broadcast([128, HEADS, HID]), op=ALU.mult)
                    t1 = sb.tile([128, HC], BF16, tag="t1")
                    nc.vector.tensor_tensor(out=t1[:], in0=go[:], in1=aB, op=ALU.mult)
                    t2 = sb.tile([128, HC], BF16, tag="t2")
                    nc.vector.tensor_tensor(out=t2[:], in0=t1[:], in1=bB, op=ALU.add)
                    if residual is None:
                        # h = relu(t2) -> h_res
                        nc.vector.tensor_scalar(out=h_res[:, b, :], in0=t2[:],
                                                scalar1=0.0, scalar2=None, op0=ALU.max)
                        hsrc = h_res[:, b, :]
                    else:
                        hr = sb.tile([128, HC], BF16, tag="hr")
                        nc.vector.tensor_scalar(out=hr[:], in0=t2[:],
                                                scalar1=0.0, scalar2=None, op0=ALU.max)
                        h2t = sb.tile([128, HC], BF16, tag="h2t")
                        nc.vector.tensor_tensor(out=h2t[:], in0=hr[:],
                                                in1=h_res[:, b, :], op=ALU.add)
                        hsrc = h2t[:]
                    for hf in range(2):
                        tp = psn.tile([128, 128], BF16, tag="tp", space="PSUM")
                        nc.tensor.transpose(tp[:], hsrc[:, hf * 128:(hf + 1) * 128],
                                            ident[:])
                        hTt = hT0 if hf == 0 else hT1
                        nc.scalar.copy(hTt[:, b * 128:b * 128 + 128], tp[:])

                for (h, t0, cn) in xl_chunks:
                    g = gat.tile([128, GMAX, HC], BF16, tag="gxl")
                    src_ap = xl_full[0:HALFN, :] if h == 0 else xl_full[HALFN:N, :]
                    gather(g[:, :cn, :], src_ap, ixl_t, t0, cn, HC)
                    gxr = gat.tile([128, GMAX, HC], BF16, tag="gxr")
                    gather(gxr[:, :cn, :], xr_tab[:], ixr_t, t0, cn, HC)

                    # compute for this chunk
                    u = sb.tile([128, GMAX, HC], BF16, tag="u")
                    nc.vector.tensor_tensor(out=u[:, :cn, :], in0=g[:, :cn, :],
                                            in1=gxr[:, :cn, :], op=ALU.add)
                    v = sb.tile([128, GMAX, HC], BF16, tag="v")
                    nc.scalar.activation(v[:, :cn, :], u[:, :cn, :], AF.Prelu,
                                         alpha=NEG_SLOPE)
                    p = sb.tile([128, GMAX, HC], BF16, tag="p")
                    nc.vector.tensor_tensor(
                        out=p[:, :cn, :], in0=v[:, :cn, :],
                        in1=attB[:, None, :].to_broadcast([128, cn, HC]), op=ALU.mult)
                    lg = sb.tile([128, GMAX, HEADS], BF16, tag="lg")
                    with nc.allow_low_precision(reason="logit accum"):
                        nc.vector.tensor_reduce(
                            out=lg[:, :cn, :],
                            in_=p[:, :cn, :].rearrange("a t (h c) -> a t h c", h=HEADS),
                            axis=mybir.AxisListType.X, op=ALU.add)
                    ex = sb.tile([128, GMAX, HEADS], BF16, tag="ex")
                    nc.scalar.activation(ex[:, :cn, :], lg[:, :cn, :], AF.Exp)
                    oh = sb.tile([128, GMAX, 128], BF16, tag="oh")
                    nc.vector.tensor_tensor(
                        out=oh[:, :cn, :],
                        in0=cs("iota")[:, None, :].to_broadcast([128, cn, 128]),
                        in1=dstl_t[:, t0:t0 + cn, None].to_broadcast([128, cn, 128]),
                        op=ALU.is_equal)
                    rhs = sb.tile([128, GMAX, HEADS, HID + 1], BF16, tag="rhs")
                    nc.vector.tensor_tensor(
                        out=rhs[:, :cn, :, 0:HID],
                        in0=g[:, :cn, :].rearrange("a t (h c) -> a t h c", h=HEADS),
                        in1=ex[:, :cn, :, None].to_broadcast([128, cn, HEADS, HID]),
                        op=ALU.mult)
                    nc.scalar.copy(rhs[:, :cn, :, HID:HID + 1],
                                   ex[:, :cn, :, None])
                    for k in range(cn):
                        t = t0 + k
                        b = tile_to_block[t]
                        if b not in accs:
                            accs[b] = ps.tile([128, HEADS, HID + 1], F32, name=f"acc{b % 4}",
                                              tag=f"acc{b % 4}", space="PSUM")
                        nc.tensor.matmul(
                            accs[b][:].rearrange("a h c -> a (h c)"),
                            lhsT=oh[:, k, :],
                            rhs=rhs[:, k, :, :].rearrange("a h c -> a (h c)"),
                            start=(t == first_tile[b]), stop=(t == last_tile[b]))
                        if t == last_tile[b]:
                            post(b)
                assert not accs, f"unclosed blocks {list(accs)}"

            edge_pass(xl0_full, xr0, cs("attB0"), cs("aB0"), cs("bB0"), None, 0)

            # ---------------- L1 node prep ----------------
            def node_mm_k2(Wk0, Wk1, tab):
                for g0 in range(0, NBLK, 8):
                    gn = min(8, NBLK - g0)
                    stage = sb.tile([128, 8, HC], BF16, tag="stg", bufs=1)
                    for k in range(gn):
                        b = g0 + k
                        st = b * 128
                        szk = min(128, NPC - st)
                        p2 = psn.tile([128, HC], F32, tag="p2", space="PSUM")
                        nc.tensor.matmul(p2[:szk, :], lhsT=hT0[:, st:st + szk],
                                         rhs=cs(Wk0), start=True, stop=False)
                        nc.tensor.matmul(p2[:szk, :], lhsT=hT1[:, st:st + szk],
                                         rhs=cs(Wk1), start=False, stop=True)
                        nc.scalar.copy(stage[:szk, k, :], p2[:szk, :])
                    rows = min(8 * 128, NPC - g0 * 128)
                    if rows == 1024:
                        nc.sync.dma_start(
                            tab[g0 * 128:g0 * 128 + 1024, :].rearrange(
                                "(t p) c -> p t c", p=128), stage[:])
                    else:
                        full = rows // 128
                        for k in range(full):
                            nc.sync.dma_start(
                                tab[g0 * 128 + k * 128:g0 * 128 + (k + 1) * 128, :],
                                stage[:, k, :])
                        rem = rows - full * 128
                        if rem:
                            nc.sync.dma_start(
                                tab[g0 * 128 + full * 128:g0 * 128 + rows, :],
                                stage[:rem, full, :])

            node_mm_k2("Wl1k0", "Wl1k1", xl1_own)
            node_mm_k2("Wr1k0", "Wr1k1", xr1)

            nc.gpsimd.collective_compute(
                "AllGather", ALU.bypass, ins=[xl1_own[:]], outs=[xl1_full[:]],
                replica_groups=rg)

            edge_pass(xl1_full, xr1, cs("attB1"), cs("aB1"), cs("bB1"), h_res, 1)

            es_idx.close()
            idx2p = ctx.enter_context(tc.tile_pool(name="idx2p", bufs=1))
            ix2_t = idx2p.tile([128, 8 * TT2], I16)
            nc.sync.dma_start(ix2_t[:], idx2[:])
            msk_t = idx2p.tile([128, TT2], BF16)
            nc.sync.dma_start(msk_t[:], mask2[:])

            # ---------------- L2 node prep ----------------
            zstage = cst.tile([128, 8, 128], BF16)
            nc.vector.memset(zstage[:], 0.0)
            for g0 in range(0, NBLK, 8):
                gn = min(8, NBLK - g0)
                stage = sb.tile([128, 8, 128], BF16, tag="stg2", bufs=1)
                nc.vector.tensor_copy(stage[:], zstage[:])
                for k in range(gn):
                    b = g0 + k
                    st = b * 128
                    szk = min(128, NPC - st)
                    p2 = psn.tile([128, OUT], F32, tag="p2l2", space="PSUM")
                    nc.tensor.matmul(p2[:szk, :], lhsT=hT0[:, st:st + szk],
                                     rhs=cs("Wl2k0"), start=True, stop=False)
                    nc.tensor.matmul(p2[:szk, :], lhsT=hT1[:, st:st + szk],
                                     rhs=cs("Wl2k1"), start=False, stop=True)
                    nc.scalar.copy(stage[:szk, k, 0:OUT], p2[:szk, :])
                    p3 = psn.tile([128, OUT], F32, tag="p2l2", space="PSUM")
                    nc.tensor.matmul(p3[:szk, :], lhsT=hT0[:, st:st + szk],
                                     rhs=cs("Wr2k0"), start=True, stop=False)
                    nc.tensor.matmul(p3[:szk, :], lhsT=hT1[:, st:st + szk],
                                     rhs=cs("Wr2k1"), start=False, stop=True)
                    nc.scalar.copy(xr2_sb[:szk, b, :], p3[:szk, :])
                rows = min(8 * 128, NPC - g0 * 128)
                if rows == 1024:
                    nc.sync.dma_start(
                        xl2_own[g0 * 128:g0 * 128 + 1024, :].rearrange(
                            "(t p) c -> p t c", p=128), stage[:])
                else:
                    full = rows // 128
                    for k in range(full):
                        nc.sync.dma_start(
                            xl2_own[g0 * 128 + k * 128:g0 * 128 + (k + 1) * 128, :],
                            stage[:, k, :])
                    rem = rows - full * 128
                    if rem:
                        nc.sync.dma_start(
                            xl2_own[g0 * 128 + full * 128:g0 * 128 + rows, :],
                            stage[:rem, full, :])

            nc.gpsimd.collective_compute(
                "AllGather", ALU.bypass, ins=[xl2_own[:]], outs=[xl2_full[:]],
                replica_groups=rg)

            # ---------------- L2 edge pass (bucketed) ----------------
            numacc = {}
            denacc = {}
            t2_to_block = {}
            first2 = {}
            last2 = {}
            for b in range(NBLK):
                tiles = []
                for (t0, Sn) in blk2_range[b]:
                    tiles += list(range(t0, t0 + Sn))
                first2[b] = min(tiles)
                last2[b] = max(tiles)
                for t in tiles:
                    t2_to_block[t] = b

            def post2(b):
                na = numacc.pop(b)
                da = denacc.pop(b)
                rc2 = sb.tile([128, 1], F32, tag="rc2")
                nc.vector.reciprocal(rc2[:], da[:])
                nc.vector.tensor_scalar(out=o_all[:, b, :], in0=na[:],
                                        scalar1=rc2[:], scalar2=None, op0=ALU.mult)

            for (h, t0, cn) in l2_chunks:
                g2 = gat.tile([128, GMAX, 128], BF16, tag="g2")
                src_ap = xl2_full[0:HALFN, :] if h == 0 else xl2_full[HALFN:N, :]
                gather(g2[:, :cn, :], src_ap, ix2_t, t0, cn, 128)
                # per-block segments within this chunk
                segs = []
                s = 0
                while s < cn:
                    b = t2_to_block[t0 + s]
                    e = s
                    while e < cn and t2_to_block[t0 + e] == b:
                        e += 1
                    segs.append((b, s, e))
                    s = e
                u2 = sb.tile([128, GMAX, OUT], BF16, tag="u2")
                for (b, s, e) in segs:
                    nc.vector.tensor_tensor(
                        out=u2[:, s:e, :], in0=g2[:, s:e, 0:OUT],
                        in1=xr2_sb[:, b, None, :].to_broadcast([128, e - s, OUT]),
                        op=ALU.add)
                v2 = sb.tile([128, GMAX, OUT], BF16, tag="v2")
                nc.scalar.activation(v2[:, :cn, :], u2[:, :cn, :], AF.Prelu,
                                     alpha=NEG_SLOPE)
                p2_ = sb.tile([128, GMAX, OUT], BF16, tag="p2_")
                nc.vector.tensor_tensor(
                    out=p2_[:, :cn, :], in0=v2[:, :cn, :],
                    in1=cs("attB2")[:, None, :].to_broadcast([128, cn, OUT]),
                    op=ALU.mult)
                lg2 = sb.tile([128, GMAX], BF16, tag="lg2")
                with nc.allow_low_precision(reason="logit accum"):
                    nc.vector.tensor_reduce(out=lg2[:, :cn], in_=p2_[:, :cn, :],
                                            axis=mybir.AxisListType.X, op=ALU.add)
                lgm = sb.tile([128, GMAX], BF16, tag="lgm")
                nc.vector.tensor_tensor(out=lgm[:, :cn], in0=lg2[:, :cn],
                                        in1=msk_t[:, t0:t0 + cn], op=ALU.add)
                ex2 = sb.tile([128, GMAX], BF16, tag="ex2")
                nc.scalar.activation(ex2[:, :cn], lgm[:, :cn], AF.Exp)
                r2 = sb.tile([128, GMAX, OUT], BF16, tag="r2")
                nc.vector.tensor_tensor(
                    out=r2[:, :cn, :], in0=g2[:, :cn, 0:OUT],
                    in1=ex2[:, :cn, None].to_broadcast([128, cn, OUT]), op=ALU.mult)
                for (b, s, e) in segs:
                    if b not in numacc:
                        numacc[b] = sb.tile([128, OUT], F32, name=f"na{b % 10}", tag=f"na{b % 10}")
                        denacc[b] = sb.tile([128, 1], F32, name=f"da{b % 10}", tag=f"da{b % 10}")
                        nc.vector.memset(numacc[b][:], 0.0)
                        nc.vector.memset(denacc[b][:], 0.0)
                    nr = sb.tile([128, OUT], F32, tag="nr")
                    nc.vector.tensor_reduce(
                        out=nr[:], in_=r2[:, s:e, :].rearrange("a t c -> a c t"),
                        axis=mybir.AxisListType.X, op=ALU.add)
                    nc.vector.tensor_tensor(out=numacc[b][:], in0=numacc[b][:],
                                            in1=nr[:], op=ALU.add)
                    dr = sb.tile([128, 1], F32, tag="dr")
                    nc.vector.tensor_reduce(out=dr[:], in_=ex2[:, s:e],
                                            axis=mybir.AxisListType.X, op=ALU.add)
                    nc.vector.tensor_tensor(out=denacc[b][:], in0=denacc[b][:],
                                            in1=dr[:], op=ALU.add)
                    if t0 + e - 1 == last2[b]:
                        post2(b)
            assert not numacc, f"unclosed l2 blocks {list(numacc)}"

            # ---------------- final: bias + log_softmax (batched) ----------------
            ob = sb.tile([128, NBLK, OUT], F32, tag="ob")
            nc.vector.tensor_tensor(
                out=ob[:], in0=o_all[:],
                in1=cs("bias2B")[:, None, :].to_broadcast([128, NBLK, OUT]),
                op=ALU.add)
            dd = sb.tile([128, NBLK], F32, tag="dd")
            nc.vector.tensor_tensor(out=dd[:], in0=ob[:, :, 1], in1=ob[:, :, 0],
                                    op=ALU.subtract)
            ee = sb.tile([128, NBLK], F32, tag="ee")
            nc.scalar.activation(ee[:], dd[:], AF.Exp)
            ep1 = sb.tile([128, NBLK], F32, tag="ep1")
            nc.vector.tensor_scalar(out=ep1[:], in0=ee[:], scalar1=1.0, scalar2=None,
                                    op0=ALU.add)
            ll = sb.tile([128, NBLK], F32, tag="ll")
            nc.scalar.activation(ll[:], ep1[:], AF.Ln)
            ls = sb.tile([128, NBLK, 2], F32, tag="ls")
            nc.vector.tensor_scalar(out=ls[:, :, 0], in0=ll[:], scalar1=-1.0,
                                    scalar2=None, op0=ALU.mult)
            nc.vector.tensor_tensor(out=ls[:, :, 1], in0=dd[:], in1=ll[:],
                                    op=ALU.subtract)
            nc.sync.dma_start(
                out[0:6144, :].rearrange("(t p) c -> p t c", p=128), ls[:, 0:48, :])
            nc.sync.dma_start(out[6144:NPC, :], ls[:106, 48, :])

    nc.compile()
    return nc


# ---------------------------------------------------------------- entry
_CACHE = {}
LAST_RESULTS = None


def kernel(**inputs):
    global COLS, LAST_RESULTS
    x = np.asarray(inputs["x"], np.float32)
    ei = np.asarray(inputs["edge_index"]).astype(np.int64)

    idx_xl, idx_xr, dstl, idx2, mask2, node_at, meta = preprocess(ei)
    carr, COLS = pack_consts(inputs)
    CW = carr.shape[1]

    key = (meta["TT"], meta["TT2"], tuple(meta["xl_chunks"]),
           tuple(meta["l2_chunks"]))
    if key not in _CACHE:
        _CACHE[key] = build(meta, CW)
    nc = _CACHE[key]

    xperm = x[node_at].astype(BF)      # [N, 128] rows in global-row order
    in_maps = []
    for c in range(NCORES):
        sl = slice(c * NPC, (c + 1) * NPC)
        cf = np.zeros((128, 2), np.float32)
        cf[:, 0] = np.arange(128)
        cf[:64, 1] = np.asarray(inputs["b_in"], np.float32)
        in_maps.append(dict(
            xT=np.ascontiguousarray(xperm[sl].T),
            idx_xl=idx_xl[c], idx_xr=idx_xr[c], dstl=dstl[c],
            idx2=idx2[c], mask2=mask2[c], consts=carr, constsF=cf,
        ))
    res = run_bass_kernel_spmd(nc, in_maps, list(range(NCORES)))
    LAST_RESULTS = res
    rows = np.concatenate([np.asarray(res.results[c]["out"]) for c in range(NCORES)],
                          axis=0).astype(np.float32)
    # rows are in global-row order; node_at[gr] = node id
    out_full = np.empty((N, OUT), np.float32)
    out_full[node_at] = rows
    return out_full


# revision 3
# speedup vs baseline: 1.0407x; 1.0407x over previous
"""Trainium2 Bass kernel for EnhancedGATModel (3-layer GATv2, N=50000, E=800000).

v2 design (8 cores, dst-partitioned, degree-sorted node permutation):
- fp16 gather tables + edge math (8x the mantissa of bf16 at identical
  2-byte DMA/DVE cost), f32 PSUM accumulation.
- L0/L1: dense edge tiles (128 edges/tile) + one-hot scatter matmuls on PE.
  Gather streams reordered [superblock: all-lo | all-hi] so xl gathers run
  in full 1024-index chunks; xr gathered from the local per-core table.
- L2: degree-bucketed (partition = dst node, slots = edges) - no one-hot,
  no xr gather, slot-reduce on DVE; pad slots masked to exp(-30).
- AllGathers in fp16; h/hT/xr2/outputs SBUF-resident; staged node-phase
  stores (1024-row DMAs); batched final log_softmax.
- dma_gather is limited to 1024 idx/op and serializes on the Pool engine
  (prepare_only/multi-queue/larger scratch all misbehave on this ucode).
"""
import sys
import numpy as np

sys.path.insert(0, "/opt/trn_rl_repo")

import concourse.bass as bass
import concourse.mybir as mybir
import concourse.tile as tile
from concourse import bacc
from concourse.bass_utils import run_bass_kernel_spmd

F32 = mybir.dt.float32
BF16 = mybir.dt.float16
I16 = mybir.dt.int16
AF = mybir.ActivationFunctionType
ALU = mybir.AluOpType
BF = np.float16

NCORES = 8
N = 50000
NPC = N // NCORES          # 6250
HALFN = N // 2             # 25000
D_IN, HID, HEADS, OUT = 128, 64, 4, 2
HC = HEADS * HID           # 256
NEG_SLOPE = 0.2
BN_EPS = 1e-5
NBLK = 49                  # 48 full 128-node blocks + one 106-node block
SB = 3                     # superblock: blocks sharing a lo/hi gather stream (PSUM-bound)
GMAX = 8                   # dma_gather limit: 1024 idx = 8 tiles of 128
PREP = False                # prepare_only + trigger_dma pipelining


# ---------------------------------------------------------------- host prep
def _wrap_idx(A):
    """[C,128,TT] (slot, tile) -> [C,128,8TT] gather index layout."""
    C, P, TT = A.shape
    B = A.transpose(0, 2, 1).reshape(C, TT, 8, 16).transpose(0, 3, 1, 2)
    return np.tile(B.reshape(C, 16, 8 * TT), (1, 8, 1)).astype(np.int16)


def preprocess(edge_index):
    src = np.concatenate([edge_index[0], np.arange(N)]).astype(np.int64)
    dst = np.concatenate([edge_index[1], np.arange(N)]).astype(np.int64)
    E2 = len(src)

    deg = np.bincount(dst, minlength=N)
    rank_order = np.argsort(-deg, kind="stable")      # rank -> node
    rr = np.empty(N, np.int64)
    rr[rank_order] = np.arange(N)                      # node -> rank
    r = np.arange(N)
    c_of_r = np.where(r < 49152, (r % 1024) // 128, (r - 49152) // 106)
    pos_of_r = np.where(r < 49152, (r // 1024) * 128 + r % 128,
                        6144 + (r - 49152) % 106)
    c_of = c_of_r[rr]                                  # node -> core
    pos_of = pos_of_r[rr]                              # node -> row in core
    gpr = c_of * NPC + pos_of                          # node -> global row
    node_at = np.empty(N, np.int64)
    node_at[gpr] = np.arange(N)                        # global row -> node

    cd = c_of[dst]
    posd = pos_of[dst]
    bd = posd // 128
    pd = posd % 128
    gs = gpr[src]
    half = (gs >= HALFN).astype(np.int64)
    sidx = gs - half * HALFN                           # int16-safe

    # ---------------- dense layout (L0/L1) ----------------
    cnt = np.zeros((NCORES, NBLK, 2), np.int64)
    np.add.at(cnt, (cd, bd, half), 1)
    mx = cnt.max(axis=0)
    T = -(-mx // 128)                                  # [NBLK,2] tiles

    order = []
    for s0 in range(0, NBLK, SB):
        bs = list(range(s0, min(s0 + SB, NBLK)))
        for h in (0, 1):
            for b in bs:
                order.append((b, h))
    tile_base = {}
    t0 = 0
    for (b, h) in order:
        tile_base[(b, h)] = t0
        t0 += T[b, h]
    TT = t0
    tb_arr = np.zeros((NBLK, 2), np.int64)
    for (b, h), v in tile_base.items():
        tb_arr[b, h] = v

    key = (cd * NBLK + bd) * 2 + half
    si = np.argsort(key, kind="stable")
    ks = key[si]
    starts = np.searchsorted(ks, np.arange(NCORES * NBLK * 2))
    rin = np.arange(E2) - starts[ks]
    cs, bs_, hs = cd[si], bd[si], half[si]
    tile_of = tb_arr[bs_, hs] + rin // 128
    slot = rin % 128

    Sxl = np.zeros((NCORES, 128, TT), np.int64)
    Sxl[cs, slot, tile_of] = sidx[si]
    Sxr = np.zeros((NCORES, 128, TT), np.int64)
    Sxr[cs, slot, tile_of] = posd[si]
    Dst = np.full((NCORES, 128, TT), 200.0, np.float32)
    Dst[cs, slot, tile_of] = pd[si]

    idx_xl = _wrap_idx(Sxl)
    idx_xr = _wrap_idx(Sxr)
    dstl = Dst.astype(BF)

    # xl gather chunks (per same-half run in stream order) + xr chunks
    xl_chunks = []
    for s0 in range(0, NBLK, SB):
        bs2 = list(range(s0, min(s0 + SB, NBLK)))
        for h in (0, 1):
            run0 = tile_base[(bs2[0], h)]
            runT = sum(int(T[b, h]) for b in bs2)
            t = run0
            while t < run0 + runT:
                n = min(GMAX, run0 + runT - t)
                xl_chunks.append((h, t, n))
                t += n
    blk_range = {b: [(tile_base[(b, 0)], int(T[b, 0])),
                     (tile_base[(b, 1)], int(T[b, 1]))] for b in range(NBLK)}

    # ---------------- bucketed layout (L2) ----------------
    key2 = key * 128 + pd
    si2 = np.argsort(key2, kind="stable")
    k2s = key2[si2]
    starts2 = np.searchsorted(k2s, np.arange(NCORES * NBLK * 2 * 128))
    rin2 = np.arange(E2) - starts2[k2s]                # slot within (c,b,h,p)
    cnt2 = np.zeros((NCORES, NBLK, 2, 128), np.int64)
    np.add.at(cnt2, (cd, bd, half, pd), 1)
    S2 = cnt2.max(axis=(0, 3))                         # [NBLK,2] slots

    base2 = {}
    t0 = 0
    for s0 in range(0, NBLK, SB):
        bs2 = list(range(s0, min(s0 + SB, NBLK)))
        for h in (0, 1):
            for b in bs2:
                base2[(b, h)] = t0
                t0 += int(S2[b, h])
    TT2 = t0
    b2_arr = np.zeros((NBLK, 2), np.int64)
    for (b, h), v in base2.items():
        b2_arr[b, h] = v

    c2, b2, h2, p2 = cd[si2], bd[si2], half[si2], pd[si2]
    tile2 = b2_arr[b2, h2] + rin2
    Sx2 = np.zeros((NCORES, 128, TT2), np.int64)
    Sx2[c2, p2, tile2] = sidx[si2]
    Msk = np.full((NCORES, 128, TT2), -30.0, np.float32)
    Msk[c2, p2, tile2] = 0.0
    idx2 = _wrap_idx(Sx2)
    mask2 = Msk.astype(BF)

    l2_chunks = []
    for s0 in range(0, NBLK, SB):
        bs2 = list(range(s0, min(s0 + SB, NBLK)))
        for h in (0, 1):
            run0 = base2[(bs2[0], h)]
            runT = sum(int(S2[b, h]) for b in bs2)
            t = run0
            while t < run0 + runT:
                n = min(GMAX, run0 + runT - t)
                l2_chunks.append((h, t, n))
                t += n
    blk2_range = {b: [(base2[(b, 0)], int(S2[b, 0])),
                      (base2[(b, 1)], int(S2[b, 1]))] for b in range(NBLK)}

    meta = dict(TT=TT, TT2=TT2, xl_chunks=xl_chunks,
                l2_chunks=l2_chunks, blk_range=blk_range, blk2_range=blk2_range,
                order=order)
    return idx_xl, idx_xr, dstl, idx2, mask2, node_at, meta


def pack_consts(ip):
    """[128, CW] bf16 const pack. Returns (array, slices dict)."""
    cols = {}
    parts = []
    c0 = [0]

    def add(name, arr):
        arr = np.asarray(arr, np.float32)
        a = np.zeros((128, arr.shape[1]), np.float32)
        a[:arr.shape[0]] = arr
        cols[name] = (arr.shape[0], c0[0], arr.shape[1])
        parts.append(a)
        c0[0] += arr.shape[1]

    bcast = lambda v: np.broadcast_to(
        np.asarray(v, np.float32).reshape(-1)[None, :], (128, np.asarray(v).size)).copy()

    iota = np.broadcast_to(np.arange(128, dtype=np.float32), (128, 128))
    add("iota", np.ascontiguousarray(iota))
    add("iotaC", np.arange(128, dtype=np.float32)[:, None])
    add("attB0", bcast(ip["att0"]))
    add("attB1", bcast(ip["att1"]))
    add("attB2", bcast(ip["att2"]))
    g, bt = np.asarray(ip["bn_gamma"]), np.asarray(ip["bn_beta"])
    mu, var = np.asarray(ip["bn_mean"]), np.asarray(ip["bn_var"])
    for l in range(2):
        a = g[l] / np.sqrt(var[l] + BN_EPS)
        b = bt[l] - mu[l] * a + a * np.asarray(ip[f"bias{l}"], np.float32)
        add(f"aB{l}", bcast(a))
        add(f"bB{l}", bcast(b))
    add("bias2B", bcast(ip["bias2"]))
    add("W_in", np.asarray(ip["W_in"], np.float32))
    add("b_in", np.asarray(ip["b_in"], np.float32).reshape(-1, 1))
    add("Wl0", np.asarray(ip["Wl0"], np.float32))
    add("Wr0", np.asarray(ip["Wr0"], np.float32))
    for nm in ("Wl1", "Wr1"):
        W = np.asarray(ip[nm], np.float32)
        add(nm + "k0", W[:128])
        add(nm + "k1", W[128:])
    for nm in ("Wl2", "Wr2"):
        W = np.asarray(ip[nm], np.float32)
        add(nm + "k0", W[:128])
        add(nm + "k1", W[128:])
    return np.concatenate(parts, axis=1).astype(BF), cols


# ---------------------------------------------------------------- device
COLS = None


def build(meta, CW):
    TT, TT2 = meta["TT"], meta["TT2"]
    xl_chunks = meta["xl_chunks"]
    l2_chunks = meta["l2_chunks"]
    blk_range, blk2_range = meta["blk_range"], meta["blk2_range"]

    nc = bacc.Bacc("TRN2", target_bir_lowering=False, debug=False,
                   num_swdge_queues=1)

    xT = nc.dram_tensor("xT", [D_IN, NPC], BF16, kind="ExternalInput")
    constsF = nc.dram_tensor("constsF", [128, 2], F32, kind="ExternalInput")
    idx_xl = nc.dram_tensor("idx_xl", [128, 8 * TT], I16, kind="ExternalInput")
    idx_xr = nc.dram_tensor("idx_xr", [128, 8 * TT], I16, kind="ExternalInput")
    dstl = nc.dram_tensor("dstl", [128, TT], BF16, kind="ExternalInput")
    idx2 = nc.dram_tensor("idx2", [128, 8 * TT2], I16, kind="ExternalInput")
    mask2 = nc.dram_tensor("mask2", [128, TT2], BF16, kind="ExternalInput")
    consts = nc.dram_tensor("consts", [128, CW], BF16, kind="ExternalInput")
    out = nc.dram_tensor("out", [NPC, OUT], F32, kind="ExternalOutput")

    xl0_own = nc.dram_tensor("xl0_own", [NPC, HC], BF16)
    xl0_full = nc.dram_tensor("xl0_full", [N, HC], BF16, addr_space="Shared")
    xr0 = nc.dram_tensor("xr0", [NPC, HC], BF16)
    xl1_own = nc.dram_tensor("xl1_own", [NPC, HC], BF16)
    xl1_full = nc.dram_tensor("xl1_full", [N, HC], BF16, addr_space="Shared")
    xr1 = nc.dram_tensor("xr1", [NPC, HC], BF16)
    xl2_own = nc.dram_tensor("xl2_own", [NPC, 128], BF16)
    xl2_full = nc.dram_tensor("xl2_full", [N, 128], BF16, addr_space="Shared")

    rg = [list(range(NCORES))]
    chunk_starts = list(range(0, NPC, 128))  # 49 chunks, last 106 wide

    with tile.TileContext(nc) as tc:
        import contextlib
        with contextlib.ExitStack() as ctx:
            cst = ctx.enter_context(tc.tile_pool(name="cst", bufs=1))
            per = ctx.enter_context(tc.tile_pool(name="per", bufs=1))
            sb = ctx.enter_context(tc.tile_pool(name="sb", bufs=2))
            gat = ctx.enter_context(tc.tile_pool(name="gat", bufs=4))
            ps = ctx.enter_context(tc.tile_pool(name="ps", bufs=1, space="PSUM"))
            psn = ctx.enter_context(tc.tile_pool(name="psn", bufs=1, space="PSUM"))

            C = cst.tile([128, CW], BF16)
            nc.sync.dma_start(C[:], consts[:])
            CF = cst.tile([128, 2], F32)
            nc.sync.dma_start(CF[:], constsF[:])
            es_idx = contextlib.ExitStack()
            idx01 = es_idx.enter_context(tc.tile_pool(name="idx01", bufs=1))
            ixl_t = idx01.tile([128, 8 * TT], I16)
            nc.sync.dma_start(ixl_t[:], idx_xl[:])
            ixr_t = idx01.tile([128, 8 * TT], I16)
            nc.sync.dma_start(ixr_t[:], idx_xr[:])
            dstl_t = idx01.tile([128, TT], BF16)
            nc.sync.dma_start(dstl_t[:], dstl[:])

            def cs(name):
                r, c0i, w = COLS[name]
                return C[0:r, c0i:c0i + w]

            ident = cst.tile([128, 128], BF16)
            nc.vector.tensor_scalar(out=ident[:], in0=cs("iota"),
                                    scalar1=CF[:, 0:1],
                                    scalar2=None, op0=ALU.is_equal)

            # persistent SBUF state
            h_res = per.tile([128, NBLK, HC], BF16)      # residual (h1)
            hT0 = per.tile([128, NBLK * 128], BF16)      # h^T channels 0:128
            hT1 = per.tile([128, NBLK * 128], BF16)      # h^T channels 128:256
            xr2_sb = per.tile([128, NBLK, OUT], BF16)    # L2 dst transform
            o_all = per.tile([128, NBLK, OUT], F32)      # L2 outputs pre-softmax

            if PREP:
                dsems = [nc.alloc_semaphore("dsem0")]
            qrr = [0]

            def gather(g_ap, table_ap, idx_tile, t0, Tc, elem, qn=None, sem=None):
                if PREP:
                    q = 0
                    nc.gpsimd.dma_gather(
                        out_ap=g_ap, in_ap=table_ap,
                        idxs_ap=idx_tile[:, 8 * t0:8 * (t0 + Tc)],
                        num_idxs=128 * Tc, num_idxs_reg=128 * Tc,
                        elem_size=elem, prepare_only=True, sem=dsems[q],
                        queue_num=q)
                    nc.gpsimd.trigger_dma(count=None, queue_num=q)
                else:
                    nc.gpsimd.dma_gather(
                        out_ap=g_ap, in_ap=table_ap,
                        idxs_ap=idx_tile[:, 8 * t0:8 * (t0 + Tc)],
                        num_idxs=128 * Tc, num_idxs_reg=128 * Tc,
                        elem_size=elem)

            # ---------------- phase A: L0 node prep ----------------
            def flush_stage(tab, stage, g0, width):
                rows = min(8 * 128, NPC - g0 * 128)
                if rows == 1024:
                    nc.sync.dma_start(
                        tab[g0 * 128:g0 * 128 + 1024, :].rearrange(
                            "(t p) c -> p t c", p=128), stage[:])
                else:
                    full = rows // 128
                    for k in range(full):
                        nc.sync.dma_start(
                            tab[g0 * 128 + k * 128:g0 * 128 + (k + 1) * 128, :],
                            stage[:, k, :])
                    rem = rows - full * 128
                    if rem:
                        nc.sync.dma_start(
                            tab[g0 * 128 + full * 128:g0 * 128 + rows, :],
                            stage[:rem, full, :])

            for g0 in range(0, NBLK, 8):
                gn = min(8, NBLK - g0)
                stageL = sb.tile([128, 8, HC], BF16, tag="stgL", bufs=1)
                stageR = sb.tile([128, 8, HC], BF16, tag="stgR", bufs=1)
                for k in range(gn):
                    st = (g0 + k) * 128
                    szk = min(128, NPC - st)
                    xTc = sb.tile([D_IN, 128], BF16, tag="xTc")
                    nc.sync.dma_start(xTc[:, :szk], xT[:, st:st + szk])
                    p1 = psn.tile([64, 128], F32, tag="p1", space="PSUM")
                    nc.tensor.matmul(p1[:, :szk], lhsT=cs("W_in"), rhs=xTc[:, :szk],
                                     start=True, stop=True)
                    h0T = sb.tile([64, 128], BF16, tag="h0T")
                    nc.scalar.activation(h0T[:, :szk], p1[:, :szk], AF.Relu,
                                         bias=CF[0:64, 1:2])
                    for W, stg in (("Wl0", stageL), ("Wr0", stageR)):
                        p2 = psn.tile([128, HC], F32, tag="p2", space="PSUM")
                        nc.tensor.matmul(p2[:szk, :], lhsT=h0T[:, :szk],
                                         rhs=cs(W), start=True, stop=True)
                        nc.scalar.copy(stg[:szk, k, :], p2[:szk, :])
                flush_stage(xl0_own, stageL, g0, HC)
                flush_stage(xr0, stageR, g0, HC)

            nc.gpsimd.collective_compute(
                "AllGather", ALU.bypass, ins=[xl0_own[:]], outs=[xl0_full[:]],
                replica_groups=rg)

            # ---------------- dense edge pass (L0/L1) ----------------
            def edge_pass(xl_full, xr_tab, attB, aB, bB, residual, layer):
                # matmul bookkeeping per block
                first_tile = {}
                last_tile = {}
                for b in range(NBLK):
                    (l0, lT), (h0_, hT_) = blk_range[b]
                    tiles = list(range(l0, l0 + lT)) + list(range(h0_, h0_ + hT_))
                    if tiles:
                        first_tile[b] = min(tiles)
                        # stop on the hi-run end (hi comes later in stream)
                        last_tile[b] = max(tiles)
                tile_to_block = {}
                for b in range(NBLK):
                    for (t0, Tn) in blk_range[b]:
                        for t in range(t0, t0 + Tn):
                            tile_to_block[t] = b
                accs = {}

                def post(b):
                    acc = accs.pop(b)
                    st = b * 128
                    nreal = min(128, NPC - st)
                    rc = sb.tile([128, HEADS, 1], F32, tag="rc")
                    nc.vector.reciprocal(rc[:], acc[:, :, HID:HID + 1])
                    go = sb.tile([128, HC], BF16, tag="go")
                    nc.vector.tensor_tensor(
                        out=go[:].rearrange("a (h c) -> a h c", h=HEADS),
                        in0=acc[:, :, 0:HID],
                        in1=rc[:].to_broadcast([128, HEADS, HID]), op=ALU.mult)
                    t1 = sb.tile([128, HC], BF16, tag="t1")
                    nc.vector.tensor_tensor(out=t1[:], in0=go[:], in1=aB, op=ALU.mult)
                    t2 = sb.tile([128, HC], BF16, tag="t2")
                    nc.vector.tensor_tensor(out=t2[:], in0=t1[:], in1=bB, op=ALU.add)
                    if residual is None:
                        # h = relu(t2) -> h_res
                        nc.vector.tensor_scalar(out=h_res[:, b, :], in0=t2[:],
                                                scalar1=0.0, scalar2=None, op0=ALU.max)
                        hsrc = h_res[:, b, :]
                    else:
                        hr = sb.tile([128, HC], BF16, tag="hr")
                        nc.vector.tensor_scalar(out=hr[:], in0=t2[:],
                                                scalar1=0.0, scalar2=None, op0=ALU.max)
                        h2t = sb.tile([128, HC], BF16, tag="h2t")
                        nc.vector.tensor_tensor(out=h2t[:], in0=hr[:],
                                                in1=h_res[:, b, :], op=ALU.add)
                        hsrc = h2t[:]
                    for hf in range(2):
                        tp = psn.tile([128, 128], BF16, tag="tp", space="PSUM")
                        nc.tensor.transpose(tp[:], hsrc[:, hf * 128:(hf + 1) * 128],
                                            ident[:])
                        hTt = hT0 if hf == 0 else hT1
                        nc.scalar.copy(hTt[:, b * 128:b * 128 + 128], tp[:])

                for (h, t0, cn) in xl_chunks:
                    g = gat.tile([128, GMAX, HC], BF16, tag="gxl")
                    src_ap = xl_full[0:HALFN, :] if h == 0 else xl_full[HALFN:N, :]
                    gather(g[:, :cn, :], src_ap, ixl_t, t0, cn, HC)
                    gxr = gat.tile([128, GMAX, HC], BF16, tag="gxr")
                    gather(gxr[:, :cn, :], xr_tab[:], ixr_t, t0, cn, HC)

                    # compute for this chunk
                    u = sb.tile([128, GMAX, HC], BF16, tag="u")
                    nc.vector.tensor_tensor(out=u[:, :cn, :], in0=g[:, :cn, :],
                                            in1=gxr[:, :cn, :], op=ALU.add)
                    v = sb.tile([128, GMAX, HC], BF16, tag="v")
                    nc.scalar.activation(v[:, :cn, :], u[:, :cn, :], AF.Prelu,
                                         alpha=NEG_SLOPE)
                    p = sb.tile([128, GMAX, HC], BF16, tag="p")
                    nc.vector.tensor_tensor(
                        out=p[:, :cn, :], in0=v[:, :cn, :],
                        in1=attB[:, None, :].to_broadcast([128, cn, HC]), op=ALU.mult)
                    lg = sb.tile([128, GMAX, HEADS], BF16, tag="lg")
                    with nc.allow_low_precision(reason="logit accum"):
                        nc.vector.tensor_reduce(
                            out=lg[:, :cn, :],
                            in_=p[:, :cn, :].rearrange("a t (h c) -> a t h c", h=HEADS),
                            axis=mybir.AxisListType.X, op=ALU.add)
                    ex = sb.tile([128, GMAX, HEADS], BF16, tag="ex")
                    nc.scalar.activation(ex[:, :cn, :], lg[:, :cn, :], AF.Exp)
                    oh = sb.tile([128, GMAX, 128], BF16, tag="oh")
                    nc.vector.tensor_tensor(
                        out=oh[:, :cn, :],
                        in0=cs("iota")[:, None, :].to_broadcast([128, cn, 128]),
                        in1=dstl_t[:, t0:t0 + cn, None].to_broadcast([128, cn, 128]),
                        op=ALU.is_equal)
                    rhs = sb.tile([128, GMAX, HEADS, HID + 1], BF16, tag="rhs")
                    nc.vector.tensor_tensor(
                        out=rhs[:, :cn, :, 0:HID],
                        in0=g[:, :cn, :].rearrange("a t (h c) -> a t h c", h=HEADS),
                        in1=ex[:, :cn, :, None].to_broadcast([128, cn, HEADS, HID]),
                        op=ALU.mult)
                    nc.scalar.copy(rhs[:, :cn, :, HID:HID + 1],
                                   ex[:, :cn, :, None])
                    for k in range(cn):
                        t = t0 + k
                        b = tile_to_block[t]
                        if b not in accs:
                            accs[b] = ps.tile([128, HEADS, HID + 1], F32, name=f"acc{b % 4}",
                                              tag=f"acc{b % 4}", space="PSUM")
                        nc.tensor.matmul(
                            accs[b][:].rearrange("a h c -> a (h c)"),
                            lhsT=oh[:, k, :],
                            rhs=rhs[:, k, :, :].rearrange("a h c -> a (h c)"),
                            start=(t == first_tile[b]), stop=(t == last_tile[b]))
                        if t == last_tile[b]:
                            post(b)
                assert not accs, f"unclosed blocks {list(accs)}"

            edge_pass(xl0_full, xr0, cs("attB0"), cs("aB0"), cs("bB0"), None, 0)

            # ---------------- L1 node prep ----------------
            def node_mm_k2(Wk0, Wk1, tab):
                for g0 in range(0, NBLK, 8):
                    gn = min(8, NBLK - g0)
                    stage = sb.tile([128, 8, HC], BF16, tag="stg", bufs=1)
                    for k in range(gn):
                        b = g0 + k
                        st = b * 128
                        szk = min(128, NPC - st)
                        p2 = psn.tile([128, HC], F32, tag="p2", space="PSUM")
                        nc.tensor.matmul(p2[:szk, :], lhsT=hT0[:, st:st + szk],
                                         rhs=cs(Wk0), start=True, stop=False)
                        nc.tensor.matmul(p2[:szk, :], lhsT=hT1[:, st:st + szk],
                                         rhs=cs(Wk1), start=False, stop=True)
                        nc.scalar.copy(stage[:szk, k, :], p2[:szk, :])
                    rows = min(8 * 128, NPC - g0 * 128)
                    if rows == 1024:
                        nc.sync.dma_start(
                            tab[g0 * 128:g0 * 128 + 1024, :].rearrange(
                                "(t p) c -> p t c", p=128), stage[:])
                    else:
                        full = rows // 128
                        for k in range(full):
                            nc.sync.dma_start(
                                tab[g0 * 128 + k * 128:g0 * 128 + (k + 1) * 128, :],
                                stage[:, k, :])
                        rem = rows - full * 128
                        if rem:
                            nc.sync.dma_start(
                                tab[g0 * 128 + full * 128:g0 * 128 + rows, :],
                                stage[:rem, full, :])

            node_mm_k2("Wl1k0", "Wl1k1", xl1_own)
            node_mm_k2("Wr1k0", "Wr1k1", xr1)

            nc.gpsimd.collective_compute(
                "AllGather", ALU.bypass, ins=[xl1_own[:]], outs=[xl1_full[:]],
                replica_groups=rg)

            edge_pass(xl1_full, xr1, cs("attB1"), cs("aB1"), cs("bB1"), h_res, 1)

            es_idx.close()
            idx2p = ctx.enter_context(tc.tile_pool(name="idx2p", bufs=1))
            ix2_t = idx2p.tile([128, 8 * TT2], I16)
            nc.sync.dma_start(ix2_t[:], idx2[:])
            msk_t = idx2p.tile([128, TT2], BF16)
            nc.sync.dma_start(msk_t[:], mask2[:])

            # ---------------- L2 node prep ----------------
            zstage = cst.tile([128, 8, 128], BF16)
            nc.vector.memset(zstage[:], 0.0)
            for g0 in range(0, NBLK, 8):
                gn = min(8, NBLK - g0)
                stage = sb.tile([128, 8, 128], BF16, tag="stg2", bufs=1)
                nc.vector.tensor_copy(stage[:], zstage[:])
                for k in range(gn):
                    b = g0 + k
                    st = b * 128
                    szk = min(128, NPC - st)
                    p2 = psn.tile([128, OUT], F32, tag="p2l2", space="PSUM")
                    nc.tensor.matmul(p2[:szk, :], lhsT=hT0[:, st:st + szk],
                                     rhs=cs("Wl2k0"), start=True, stop=False)
                    nc.tensor.matmul(p2[:szk, :], lhsT=hT1[:, st:st + szk],
                                     rhs=cs("Wl2k1"), start=False, stop=True)
                    nc.scalar.copy(stage[:szk, k, 0:OUT], p2[:szk, :])
                    p3 = psn.tile([128, OUT], F32, tag="p2l2", space="PSUM")
                    nc.tensor.matmul(p3[:szk, :], lhsT=hT0[:, st:st + szk],
                                     rhs=cs("Wr2k0"), start=True, stop=False)
                    nc.tensor.matmul(p3[:szk, :], lhsT=hT1[:, st:st + szk],
                                     rhs=cs("Wr2k1"), start=False, stop=True)
                    nc.scalar.copy(xr2_sb[:szk, b, :], p3[:szk, :])
                rows = min(8 * 128, NPC - g0 * 128)
                if rows == 1024:
                    nc.sync.dma_start(
                        xl2_own[g0 * 128:g0 * 128 + 1024, :].rearrange(
                            "(t p) c -> p t c", p=128), stage[:])
                else:
                    full = rows // 128
                    for k in range(full):
                        nc.sync.dma_start(
                            xl2_own[g0 * 128 + k * 128:g0 * 128 + (k + 1) * 128, :],
                            stage[:, k, :])
                    rem = rows - full * 128
                    if rem:
                        nc.sync.dma_start(
                            xl2_own[g0 * 128 + full * 128:g0 * 128 + rows, :],
                            stage[:rem, full, :])

            nc.gpsimd.collective_compute(
                "AllGather", ALU.bypass, ins=[xl2_own[:]], outs=[xl2_full[:]],
                replica_groups=rg)

            # ---------------- L2 edge pass (bucketed) ----------------
            numacc = {}
            denacc = {}
            t2_to_block = {}
            first2 = {}
            last2 = {}
            for b in range(NBLK):
                tiles = []
                for (t0, Sn) in blk2_range[b]:
                    tiles += list(range(t0, t0 + Sn))
                first2[b] = min(tiles)
                last2[b] = max(tiles)
                for t in tiles:
                    t2_to_block[t] = b

            def post2(b):
                na = numacc.pop(b)
                da = denacc.pop(b)
                rc2 = sb.tile([128, 1], F32, tag="rc2")
                nc.vector.reciprocal(rc2[:], da[:])
                nc.vector.tensor_scalar(out=o_all[:, b, :], in0=na[:],
                                        scalar1=rc2[:], scalar2=None, op0=ALU.mult)

            for (h, t0, cn) in l2_chunks:
                g2 = gat.tile([128, GMAX, 128], BF16, tag="g2")
                src_ap = xl2_full[0:HALFN, :] if h == 0 else xl2_full[HALFN:N, :]
                gather(g2[:, :cn, :], src_ap, ix2_t, t0, cn, 128)
                # per-block segments within this chunk
                segs = []
                s = 0
                while s < cn:
                    b = t2_to_block[t0 + s]
                    e = s
                    while e < cn and t2_to_block[t0 + e] == b:
                        e += 1
                    segs.append((b, s, e))
                    s = e
                u2 = sb.tile([128, GMAX, OUT], BF16, tag="u2")
                for (b, s, e) in segs:
                    nc.vector.tensor_tensor(
                        out=u2[:, s:e, :], in0=g2[:, s:e, 0:OUT],
                        in1=xr2_sb[:, b, None, :].to_broadcast([128, e - s, OUT]),
                        op=ALU.add)
                v2 = sb.tile([128, GMAX, OUT], BF16, tag="v2")
                nc.scalar.activation(v2[:, :cn, :], u2[:, :cn, :], AF.Prelu,
                                     alpha=NEG_SLOPE)
                p2_ = sb.tile([128, GMAX, OUT], BF16, tag="p2_")
                nc.vector.tensor_tensor(
                    out=p2_[:, :cn, :], in0=v2[:, :cn, :],
                    in1=cs("attB2")[:, None, :].to_broadcast([128, cn, OUT]),
                    op=ALU.mult)
                lg2 = sb.tile([128, GMAX], BF16, tag="lg2")
                with nc.allow_low_precision(reason="logit accum"):
                    nc.vector.tensor_reduce(out=lg2[:, :cn], in_=p2_[:, :cn, :],
                                            axis=mybir.AxisListType.X, op=ALU.add)
                lgm = sb.tile([128, GMAX], BF16, tag="lgm")
                nc.vector.tensor_tensor(out=lgm[:, :cn], in0=lg2[:, :cn],
                                        in1=msk_t[:, t0:t0 + cn], op=ALU.add)
                ex2 = sb.tile([128, GMAX], BF16, tag="ex2")
                nc.scalar.activation(ex2[:, :cn], lgm[:, :cn], AF.Exp)
                r2 = sb.tile([128, GMAX, OUT], BF16, tag="r2")
                nc.vector.tensor_tensor(
                    out=r2[:, :cn, :], in0=g2[:, :cn, 0:OUT],
                    in1=ex2[:, :cn, None].to_broadcast([128, cn, OUT]), op=ALU.mult)
                for (b, s, e) in segs:
                    if b not in numacc:
                        numacc[b] = sb.tile([128, OUT], F32, name=f"na{b % 10}", tag=f"na{b % 10}")
                        denacc[b] = sb.tile([128, 1], F32, name=f"da{b % 10}", tag=f"da{b % 10}")
                        nc.vector.memset(numacc[b][:], 0.0)
                        nc.vector.memset(denacc[b][:], 0.0)
                    nr = sb.tile([128, OUT], F32, tag="nr")
                    nc.vector.tensor_reduce(
                        out=nr[:], in_=r2[:, s:e, :].rearrange("a t c -> a c t"),
                        axis=mybir.AxisListType.X, op=ALU.add)
                    nc.vector.tensor_tensor(out=numacc[b][:], in0=numacc[b][:],
                                            in1=nr[:], op=ALU.add)
                    dr = sb.tile([128, 1], F32, tag="dr")
                    nc.vector.tensor_reduce(out=dr[:], in_=ex2[:, s:e],
                                            axis=mybir.AxisListType.X, op=ALU.add)
                    nc.vector.tensor_tensor(out=denacc[b][:], in0=denacc[b][:],
                                            in1=dr[:], op=ALU.add)
                    if t0 + e - 1 == last2[b]:
                        post2(b)
            assert not numacc, f"unclosed l2 blocks {list(numacc)}"

            # ---------------- final: bias + log_softmax (batched) ----------------
            ob = sb.tile([128, NBLK, OUT], F32, tag="ob")
            nc.vector.tensor_tensor(
                out=ob[:], in0=o_all[:],
                in1=cs("bias2B")[:, None, :].to_broadcast([128, NBLK, OUT]),
                op=ALU.add)
            dd = sb.tile([128, NBLK], F32, tag="dd")
            nc.vector.tensor_tensor(out=dd[:], in0=ob[:, :, 1], in1=ob[:, :, 0],
                                    op=ALU.subtract)
            ee = sb.tile([128, NBLK], F32, tag="ee")
            nc.scalar.activation(ee[:], dd[:], AF.Exp)
            ep1 = sb.tile([128, NBLK], F32, tag="ep1")
            nc.vector.tensor_scalar(out=ep1[:], in0=ee[:], scalar1=1.0, scalar2=None,
                                    op0=ALU.add)
            ll = sb.tile([128, NBLK], F32, tag="ll")
            nc.scalar.activation(ll[:], ep1[:], AF.Ln)
            ls = sb.tile([128, NBLK, 2], F32, tag="ls")
            nc.vector.tensor_scalar(out=ls[:, :, 0], in0=ll[:], scalar1=-1.0,
                                    scalar2=None, op0=ALU.mult)
            nc.vector.tensor_tensor(out=ls[:, :, 1], in0=dd[:], in1=ll[:],
                                    op=ALU.subtract)
            nc.sync.dma_start(
                out[0:6144, :].rearrange("(t p) c -> p t c", p=128), ls[:, 0:48, :])
            nc.sync.dma_start(out[6144:NPC, :], ls[:106, 48, :])

    nc.compile()
    return nc


# ---------------------------------------------------------------- entry
_CACHE = {}
LAST_RESULTS = None


def kernel(**inputs):
    global COLS, LAST_RESULTS
    x = np.asarray(inputs["x"], np.float32)
    ei = np.asarray(inputs["edge_index"]).astype(np.int64)

    idx_xl, idx_xr, dstl, idx2, mask2, node_at, meta = preprocess(ei)
    carr, COLS = pack_consts(inputs)
    CW = carr.shape[1]

    key = (meta["TT"], meta["TT2"], tuple(meta["xl_chunks"]),
           tuple(meta["l2_chunks"]))
    if key not in _CACHE:
        _CACHE[key] = build(meta, CW)
    nc = _CACHE[key]

    xperm = x[node_at].astype(BF)      # [N, 128] rows in global-row order
    in_maps = []
    for c in range(NCORES):
        sl = slice(c * NPC, (c + 1) * NPC)
        cf = np.zeros((128, 2), np.float32)
        cf[:, 0] = np.arange(128)
        cf[:64, 1] = np.asarray(inputs["b_in"], np.float32)
        in_maps.append(dict(
            xT=np.ascontiguousarray(xperm[sl].T),
            idx_xl=idx_xl[c], idx_xr=idx_xr[c], dstl=dstl[c],
            idx2=idx2[c], mask2=mask2[c], consts=carr, constsF=cf,
        ))
    res = run_bass_kernel_spmd(nc, in_maps, list(range(NCORES)))
    LAST_RESULTS = res
    rows = np.concatenate([np.asarray(res.results[c]["out"]) for c in range(NCORES)],
                          axis=0).astype(np.float32)
    # rows are in global-row order; node_at[gr] = node id
    out_full = np.empty((N, OUT), np.float32)
    out_full[node_at] = rows
    return out_full
